# revision 4
# baseline (speedup 1.0000x reference)
"""Trainium2 Bass kernel for CombinedRegistrationLoss.

Math (per batch b, B=8, N=M=4096):
  pred_src = (source_h @ pred_T^T)[:, :3]   (host, fp32)
  gt_src   = (source_h @ gt_T^T)[:, :3]     (host, fp32)
  chamferA = chamfer(pred_src, target)      (device)
  chamferB = chamfer(pred_src, gt_src)      (device)
  transform loss: frobenius/vector norms    (host, tiny)

Device strategy (pure data parallel, 1 batch per NeuronCore):
  dist[n,m] = |x_n|^2 + |y_m|^2 - 2 x.y is computed as ONE K=16 matmul per
  (128 x) x (512 y) tile: fp32 values are split into bf16 hi+lo pairs so the
  PE runs at bf16 rate (1 cyc/row) while keeping ~fp16-level precision in the
  fp32 PSUM accumulation.  Per x-tile [128, 2048]-wide PSUM halves are:
    - copied+rounded to bf16 SBUF by the Scalar engine (ACT),
    - column-min accumulated across x-tiles by DVE tensor_tensor(min)
      (bf16 SBUF -> 2x mode),
    - row-min reduced by a single fused tensor_tensor_reduce per x-tile.
  Column partials [128, 4096] are partition-min-reduced via PE transposes +
  one 3D-AP tensor_reduce.  Each core outputs a [128, 128] f32 tile of
  row/col minima; the host averages (cheap, exact).
"""

import os
from contextlib import ExitStack

import numpy as np
import ml_dtypes

BF16_NP = ml_dtypes.bfloat16

# problem constants (hardcoded per harness contract)
B = 8
NPTS = 4096          # points per cloud
N_CORES = 8
PSUM_W = 2048        # psum half-tile width
XT = NPTS // 128     # 32 x-tiles
NH = NPTS // PSUM_W  # 2 psum halves
NJ = PSUM_W // 512   # 4 matmuls per half
NB = PSUM_W // 128   # 16 transpose blocks per half

_CACHE = {}
LAST_RESULTS = None  # BassKernelResults of the most recent device run


def _build_bass(npts=NPTS, psum_w=PSUM_W):
    import concourse.bass as bass  # noqa: F401
    import concourse.tile as tile
    from concourse import bacc, mybir

    F32 = mybir.dt.float32
    BF16 = mybir.dt.bfloat16
    MIN = mybir.AluOpType.min
    X = mybir.AxisListType.X
    BIG = 3.0e38

    xt = npts // 128
    nh = max(1, npts // psum_w)
    w = min(npts, psum_w)
    nj = w // 512
    nb = w // 128

    nc = bacc.Bacc(
        "TRN2",
        target_bir_lowering=False,
        debug=False,
        enable_asserts=False,
        num_devices=N_CORES,
    )

    packs = nc.dram_tensor("packs", [16, 3 * npts], BF16, kind="ExternalInput")
    ident = nc.dram_tensor("ident", [128, 128], BF16, kind="ExternalInput")
    out = nc.dram_tensor("out", [128, 4 * xt], F32, kind="ExternalOutput")

    with tile.TileContext(nc) as tc:
        with ExitStack() as ctx:
            const_pool = ctx.enter_context(tc.tile_pool(name="const", bufs=1))
            acc_pool = ctx.enter_context(tc.tile_pool(name="acc", bufs=2))
            t_pool = ctx.enter_context(tc.tile_pool(name="t", bufs=4))
            dummy_pool = ctx.enter_context(tc.tile_pool(name="dummy", bufs=4))
            racc_pool = ctx.enter_context(tc.tile_pool(name="racc", bufs=2))
            ps_pool = ctx.enter_context(
                tc.tile_pool(name="ps", bufs=2, space="PSUM")
            )

            packs_s = const_pool.tile([16, 3 * npts], BF16)
            nc.sync.dma_start(packs_s[:, :], packs.ap()[:, :])
            ident_s = const_pool.tile([128, 128], BF16)
            nc.sync.dma_start(ident_s[:, :], ident.ap()[:, :])
            res_s = const_pool.tile([128, 4 * xt], F32)

            xp = packs_s[:, 0:npts]
            yps = [packs_s[:, npts : 2 * npts], packs_s[:, 2 * npts : 3 * npts]]

            for mat in range(2):
                yp = yps[mat]
                base = 2 * xt * mat
                # column-min accumulator over all psum halves (bf16, 2x TT mode)
                colacc = acc_pool.tile(
                    [128, nh * w], BF16, tag="acc", name=f"colacc{mat}"
                )

                for i in range(xt):
                    t = t_pool.tile([128, nh * w], BF16, tag="t", name=f"t{mat}_{i}")
                    for h in range(nh):
                        ps = ps_pool.tile([128, w], F32, tag="ps", name=f"ps{mat}_{i}_{h}")
                        for jj in range(nj):
                            nc.tensor.matmul(
                                ps[:, jj * 512 : (jj + 1) * 512],
                                xp[:, i * 128 : (i + 1) * 128],
                                yp[:, h * w + jj * 512 : h * w + (jj + 1) * 512],
                                start=True,
                                stop=True,
                            )
                        nc.scalar.copy(t[:, h * w : (h + 1) * w], ps[:, :])
                    if i == 0:
                        # 4x-mode bf16 copy seeds the accumulator (no memset needed)
                        nc.vector.tensor_copy(colacc[:, :], t[:, :])
                    else:
                        nc.vector.tensor_tensor(
                            colacc[:, :], colacc[:, :], t[:, :], op=MIN
                        )
                    # fused row-min via standard tensor_scalar accum:
                    #   accum = reduce_min(min(in0, BIG)) min scalar2
                    # chained across halves (scalar2 carries the partial).
                    racc = None
                    for h in range(nh):
                        dummy = dummy_pool.tile(
                            [128, w], BF16, tag="dummy", name=f"dm{mat}_{i}_{h}"
                        )
                        last = h == nh - 1
                        acc_dst = (
                            res_s[:, base + i : base + i + 1]
                            if last
                            else racc_pool.tile(
                                [128, 1], F32, tag="racc", name=f"ra{mat}_{i}_{h}"
                            )[:, :]
                        )
                        nc.vector.tensor_scalar(
                            out=dummy[:, :],
                            in0=t[:, h * w : (h + 1) * w],
                            scalar1=BIG,
                            scalar2=racc,
                            op0=MIN,
                            op1=MIN,
                            accum_out=acc_dst,
                        )
                        racc = acc_dst

                # partition-min of colacc via PE transpose + 3D reduce
                for h in range(nh):
                    pst = ps_pool.tile([128, w], BF16, tag="ps", name=f"pst{mat}_{h}")
                    for jb in range(nb):
                        nc.tensor.transpose(
                            pst[:, jb * 128 : (jb + 1) * 128],
                            colacc[:, h * w + jb * 128 : h * w + (jb + 1) * 128],
                            ident_s[:, :],
                        )
                    c0 = base + xt + h * nb
                    nc.vector.tensor_reduce(
                        out=res_s[:, c0 : c0 + nb],
                        in_=pst[:, :].rearrange("p (j q) -> p j q", q=128),
                        axis=X,
                        op=MIN,
                    )

            nc.sync.dma_start(out.ap()[:, :], res_s[:, :])

    nc.compile()
    return nc


def _get_nc():
    if "nc" not in _CACHE:
        _CACHE["nc"] = _build_bass()
    return _CACHE["nc"]


def _split_bf16(a):
    """fp32 -> (hi, lo) bf16 pair with hi + lo ~= a."""
    hi = a.astype(BF16_NP)
    lo = (a - hi.astype(np.float32)).astype(BF16_NP)
    return hi, lo


def _make_pack(x_f32, is_x):
    """Build the [16, n] bf16 K-pack for one point cloud.

    lhsT (x side) rows: [xh0..2, xl0..2, xh0..2, xl0..2, Xn_h, Xn_l, 1, 1]
    rhs  (y side) rows: [vh0..2, vh0..2, vl0..2, vl0..2, 1, 1, Yn_h, Yn_l]
    with v = -2*y, so that sum_k lhsT[k]*rhs[k] = |x|^2 + |y|^2 - 2 x.y.
    """
    n = x_f32.shape[0]
    pack = np.zeros((16, n), dtype=BF16_NP)
    nrm = np.sum(x_f32 * x_f32, axis=1, dtype=np.float32)
    nh, nl = _split_bf16(nrm)
    if is_x:
        h, l = _split_bf16(x_f32)
        pack[0:3] = h.T
        pack[3:6] = l.T
        pack[6:9] = h.T
        pack[9:12] = l.T
        pack[12] = nh
        pack[13] = nl
        pack[14:16] = np.ones((2, n), dtype=BF16_NP)
    else:
        v = (-2.0 * x_f32).astype(np.float32)
        h, l = _split_bf16(v)
        pack[0:3] = h.T
        pack[3:6] = h.T
        pack[6:9] = l.T
        pack[9:12] = l.T
        pack[12:14] = np.ones((2, n), dtype=BF16_NP)
        pack[14] = nh
        pack[15] = nl
    return pack


def kernel(pred_transform, gt_transform, source_points, target_points):
    global LAST_RESULTS
    from concourse import bass_utils

    pred_transform = np.asarray(pred_transform, dtype=np.float32)
    gt_transform = np.asarray(gt_transform, dtype=np.float32)
    source_points = np.asarray(source_points, dtype=np.float32)
    target_points = np.asarray(target_points, dtype=np.float32)

    b, n, _ = source_points.shape
    assert (b, n) == (B, NPTS), (b, n)

    # host: transform the source points (tiny fp32 matmuls, exact)
    src_h = np.concatenate(
        [source_points, np.ones((b, n, 1), np.float32)], axis=2
    )
    pred_src = np.einsum(
        "bnk,bjk->bnj", src_h, pred_transform, dtype=np.float32
    )[:, :, :3].astype(np.float32)
    gt_src = np.einsum(
        "bnk,bjk->bnj", src_h, gt_transform, dtype=np.float32
    )[:, :, :3].astype(np.float32)

    # per-core device inputs
    ident = np.eye(128, dtype=BF16_NP)
    in_maps = []
    for i in range(B):
        packs = np.concatenate(
            [
                _make_pack(pred_src[i], True),
                _make_pack(target_points[i], False),
                _make_pack(gt_src[i], False),
            ],
            axis=1,
        )
        in_maps.append({"packs": packs, "ident": ident})

    nc = _get_nc()
    trace = bool(int(os.environ.get("KERNEL_TRACE", "0")))
    res = bass_utils.run_bass_kernel_spmd(
        nc,
        in_maps,
        core_ids=list(range(N_CORES)),
        trace=trace,
        stitch_traces=False,
    )
    LAST_RESULTS = res

    # host: combine per-core row/col minima into the 4 loss scalars
    cham = np.zeros((2, B), dtype=np.float64)
    for i in range(B):
        r = res.results[i]["out"].astype(np.float64)  # [128, 128]
        for mat in range(2):
            rowmins = r[:, 2 * XT * mat : 2 * XT * mat + XT]
            colmins = r[:, 2 * XT * mat + XT : 2 * XT * mat + 2 * XT]
            cham[mat, i] = rowmins.mean() + colmins.mean()

    dR = (pred_transform[:, :3, :3] - gt_transform[:, :3, :3]).astype(np.float64)
    dt = (pred_transform[:, :3, 3] - gt_transform[:, :3, 3]).astype(np.float64)
    rot = np.sqrt(np.sum(dR * dR, axis=(1, 2)))
    tra = np.sqrt(np.sum(dt * dt, axis=1))
    tl = rot + tra

    total = cham[0] + tl + 0.5 * cham[1]
    out = np.array(
        [total.mean(), cham[0].mean(), tl.mean(), cham[1].mean()],
        dtype=np.float32,
    )
    return out
